# revision 8
# baseline (speedup 1.0000x reference)
"""v4: fp16, xp/xm folded into PE pass-1, minimal DVE phase A, batched tail.

Fields smoothed: {x, y, xy (taps x4 in pass1), s=x^2+y^2 (taps x2 in pass1)}.
Pass-1 accumulates xp-interm = pass1(x)+pass1(y), xm-interm = pass1(x)-pass1(y)
directly in PSUM using banded matmuls with +band / -band, so no elementwise
xp/xm ops exist at all.

Outputs per core: [sum_x2, sum_y2, sum_xy, sum_ssim, sum_bce];
host: mse_sum = sum_x2 + sum_y2 - 2*sum_xy.
"""

import sys

sys.path.insert(0, "/opt/trn_rl_repo")

import numpy as np

import concourse.bass as bass
import concourse.bacc as bacc
import concourse.mybir as mybir
from concourse.mybir import ActivationFunctionType as AF
from concourse.mybir import AluOpType as ALU
from concourse.tile import TileContext

F32 = mybir.dt.float32
F16 = mybir.dt.float16

B, C, H, W = 16, 3, 512, 512
NB = 1024
N_CORES = 8
B_LOC = B // N_CORES
N_IMG = B_LOC * C
C1 = 0.01 ** 2
C2 = 0.03 ** 2
CURRICULUM_EP = 12
LI, LS, LW = 0.5, 0.8, 3.0

OFFS = [0, 123, 251, 379]
NS = [512, 138, 138, 133]


def _gauss_1d():
    coords = np.arange(11, dtype=np.float32) - 5
    g = np.exp(-(coords ** 2) / (2 * 1.5 ** 2)).astype(np.float32)
    g = g / g.sum()
    return g.astype(np.float32)


def _tuned_f16_taps():
    """fp16 taps whose float64 sum is exactly 1 (tuned via the tiny edge
    taps); the linear/quadratic tap-sum mismatch in the SSIM covariance
    otherwise becomes a systematic bias."""
    g = _gauss_1d().astype(np.float64)
    t = g.astype(np.float16)
    for _ in range(20):
        r = 1.0 - t.astype(np.float64).sum()
        if abs(r) < 1e-9:
            break
        t[0] = np.float16(t[0] + r / 2)
        t[10] = np.float16(t[10] + r / 2)
    return t.astype(np.float32)


def _band_blocks(scale=1.0):
    """Band blocks for taps*scale (scale must be a power of two so the f16
    values stay exact)."""
    g = _tuned_f16_taps() * scale
    blocks = []
    for k in range(4):
        blk = np.zeros((128, NS[k]), dtype=np.float32)
        for r in range(128):
            h_in = 128 * k + r
            for j in range(NS[k]):
                h_out = OFFS[k] + j
                d = h_in - h_out + 5
                if 0 <= d <= 10:
                    blk[r, j] = g[d]
        blocks.append(blk.astype(np.float16))
    return blocks


# y-part of pass-1 for the "k=0" matmul only needs the tap-covered columns
NS_Y0 = 133


def _build_program(compile=True):
    nc = bacc.Bacc("TRN2", target_bir_lowering=False)

    cover = nc.declare_dram_parameter("cover", [B_LOC, C, H, W], F16, isOutput=False)
    wmed = nc.declare_dram_parameter("wmed", [B_LOC, C, H, W], F16, isOutput=False)
    wm_orig = nc.declare_dram_parameter("wm_orig", [B_LOC, NB], F32, isOutput=False)
    wm_ext = nc.declare_dram_parameter("wm_ext", [B_LOC, NB], F32, isOutput=False)
    # band sets: b1 (taps), b1n (-taps), b4 (4*taps), b2 (2*taps)
    band_names = ["b1", "b1n", "b4", "b2"]
    bands = {
        nm: [
            nc.declare_dram_parameter(f"{nm}_{k}", [128, NS[k]], F16, isOutput=False)
            for k in range(4)
        ]
        for nm in band_names
    }
    out = nc.declare_dram_parameter("out", [1, 8], F32, isOutput=True)

    c11 = 2.0 * C1
    c22 = 2.0 * C2

    with TileContext(nc) as tc:
        import contextlib

        with contextlib.ExitStack() as ctx:
            singles = ctx.enter_context(tc.tile_pool(name="singles", bufs=1))
            imgpool = ctx.enter_context(tc.tile_pool(name="img", bufs=2))
            fieldpool = ctx.enter_context(tc.tile_pool(name="field", bufs=2))
            itmpool = ctx.enter_context(tc.tile_pool(name="itm", bufs=2))
            cpool = ctx.enter_context(tc.tile_pool(name="pc", bufs=3))
            p1pool = ctx.enter_context(tc.tile_pool(name="psum1", bufs=2, space="PSUM"))
            p2pool = ctx.enter_context(tc.tile_pool(name="psum2", bufs=6, space="PSUM"))

            band_sb = {}
            for nm in band_names:
                tiles = []
                for k in range(4):
                    t = singles.tile([128, NS[k]], F16, tag=f"{nm}_{k}")
                    nc.sync.dma_start(out=t[:], in_=bands[nm][k][:])
                    tiles.append(t)
                band_sb[nm] = tiles
            ones = singles.tile([128, 1], F32, tag="ones")
            nc.vector.memset(ones[:], 1.0)

            acc_x2 = singles.tile([128, N_IMG], F32, tag="acc_x2")
            acc_y2 = singles.tile([128, N_IMG], F32, tag="acc_y2")
            acc_xy = singles.tile([128, N_IMG], F32, tag="acc_xy")
            acc_ss = singles.tile([128, 4 * N_IMG], F32, tag="acc_ss")
            acc_bce = singles.tile([128, 1], F32, tag="acc_bce")
            nc.vector.memset(acc_bce[:], 0.0)

            # ---------------- BCE ----------------
            o_t = singles.tile([B_LOC, NB], F32, tag="wmo")
            e_t = singles.tile([B_LOC, NB], F32, tag="wme")
            nc.sync.dma_start(out=o_t[:], in_=wm_orig[:])
            nc.sync.dma_start(out=e_t[:], in_=wm_ext[:])
            l1 = singles.tile([B_LOC, NB], F32, tag="l1")
            l2 = singles.tile([B_LOC, NB], F32, tag="l2")
            om = singles.tile([B_LOC, NB], F32, tag="om")
            d12 = singles.tile([B_LOC, NB], F32, tag="d12")
            m1 = singles.tile([B_LOC, NB], F32, tag="m1")
            nc.scalar.activation(l1[:], e_t[:], AF.Ln)
            nc.vector.tensor_scalar(om[:], e_t[:], -1.0, 1.0, ALU.mult, ALU.add)
            nc.scalar.activation(l2[:], om[:], AF.Ln)
            nc.vector.tensor_tensor(d12[:], l1[:], l2[:], ALU.subtract)
            nc.vector.tensor_tensor(m1[:], o_t[:], d12[:], ALU.mult)
            nc.vector.scalar_tensor_tensor(
                m1[:], m1[:], 0.0, l2[:], ALU.add, ALU.add,
                accum_out=acc_bce[:B_LOC, 0:1],
            )

            # ---------------- main loop ----------------
            for img in range(N_IMG):
                b, ch = divmod(img, C)
                x16 = fieldpool.tile([128, 2048], F16, tag="x16")
                y16 = fieldpool.tile([128, 2048], F16, tag="y16")
                x2 = fieldpool.tile([128, 2048], F16, tag="x2")
                y2 = fieldpool.tile([128, 2048], F16, tag="y2")
                xy = fieldpool.tile([128, 2048], F16, tag="xy")
                s_f = fieldpool.tile([128, 2048], F16, tag="s")
                src_x = wmed[b, ch].rearrange("(t p) w -> p t w", p=128)
                src_y = cover[b, ch].rearrange("(t p) w -> p t w", p=128)
                nc.sync.dma_start(out=x16[:].rearrange("p (t w) -> p t w", t=4), in_=src_x)
                nc.sync.dma_start(out=y16[:].rearrange("p (t w) -> p t w", t=4), in_=src_y)
                nc.scalar.activation(
                    x2[:], x16[:], AF.Square, accum_out=acc_x2[:, img : img + 1]
                )
                nc.scalar.activation(
                    y2[:], y16[:], AF.Square, accum_out=acc_y2[:, img : img + 1]
                )
                nc.vector.scalar_tensor_tensor(
                    xy[:], x16[:], 1.0, y16[:], ALU.mult, ALU.mult,
                    accum_out=acc_xy[:, img : img + 1],
                )
                nc.gpsimd.tensor_tensor(s_f[:], x2[:], y2[:], ALU.add)

                # pass 1 -> four fp16 interms: xp=(x+y), xm=(x-y), xy*4, s*2
                itm_xp = itmpool.tile([128, 2048], F16, tag="itm_xp")
                itm_xm = itmpool.tile([128, 2048], F16, tag="itm_xm")
                itm_xy = itmpool.tile([128, 2048], F16, tag="itm_xy")
                itm_s = itmpool.tile([128, 2048], F16, tag="itm_s")

                for m in range(4):
                    # xp / xm: accumulate x and +/-y matmuls in one psum tile
                    for itm, yband in ((itm_xp, "b1"), (itm_xm, "b1n")):
                        ps = p1pool.tile([128, 512], F32, tag="p1")
                        for k in range(4):
                            lhsT = x16[:, k * 512 + m * 128 : k * 512 + m * 128 + 128]
                            nc.tensor.matmul(
                                ps[:, OFFS[k] : OFFS[k] + NS[k]],
                                lhsT,
                                band_sb["b1"][k][:],
                                start=(k == 0),
                                stop=False,
                            )
                        for k in range(4):
                            lhsT = y16[:, k * 512 + m * 128 : k * 512 + m * 128 + 128]
                            n_k = NS_Y0 if k == 0 else NS[k]
                            nc.tensor.matmul(
                                ps[:, OFFS[k] : OFFS[k] + n_k],
                                lhsT,
                                band_sb[yband][k][:, 0:n_k],
                                start=False,
                                stop=(k == 3),
                            )
                        nc.any.tensor_copy(itm[:, m * 512 : (m + 1) * 512], ps[:])
                    for itm, F_t, bnm in (
                        (itm_xy, xy, "b4"),
                        (itm_s, s_f, "b2"),
                    ):
                        ps = p1pool.tile([128, 512], F32, tag="p1")
                        for k in range(4):
                            lhsT = F_t[:, k * 512 + m * 128 : k * 512 + m * 128 + 128]
                            nc.tensor.matmul(
                                ps[:, OFFS[k] : OFFS[k] + NS[k]],
                                lhsT,
                                band_sb[bnm][k][:],
                                start=(k == 0),
                                stop=(k == 3),
                            )
                        nc.any.tensor_copy(itm[:, m * 512 : (m + 1) * 512], ps[:])

                # pass 2 + per-chunk phase C
                for t in range(4):
                    sm_ps = []
                    for itm in (itm_xp, itm_xm, itm_xy, itm_s):
                        ps = p2pool.tile([128, 512], F32, tag="p2")
                        sm_ps.append(ps)
                        for k in range(4):
                            lhsT = itm[:, k * 512 + t * 128 : k * 512 + t * 128 + 128]
                            nc.tensor.matmul(
                                ps[:, OFFS[k] : OFFS[k] + NS[k]],
                                lhsT,
                                band_sb["b1"][k][:],
                                start=(k == 0),
                                stop=(k == 3),
                            )
                    sp_t, smm_t, sm4_t, ss2_t = sm_ps

                    P_t = cpool.tile([128, 512], F32, tag="P")
                    Q_t = cpool.tile([128, 512], F32, tag="Q")
                    G_t = cpool.tile([128, 512], F32, tag="G")
                    H_t = cpool.tile([128, 512], F32, tag="Hh")
                    t1_t = cpool.tile([128, 512], F16, tag="t1")
                    t2_t = cpool.tile([128, 512], F16, tag="t2")
                    num_t = cpool.tile([128, 512], F16, tag="num")
                    den_t = cpool.tile([128, 512], F32, tag="den")
                    rec_t = cpool.tile([128, 512], F32, tag="rec")
                    scr_t = cpool.tile([128, 512], F32, tag="scr")

                    nc.scalar.activation(P_t[:], sp_t[:], AF.Square)
                    nc.scalar.activation(Q_t[:], smm_t[:], AF.Square)
                    nc.gpsimd.tensor_tensor(G_t[:], P_t[:], Q_t[:], ALU.subtract)
                    nc.gpsimd.tensor_tensor(H_t[:], P_t[:], Q_t[:], ALU.add)
                    nc.vector.scalar_tensor_tensor(
                        t1_t[:], sm4_t[:], c22, G_t[:], ALU.add, ALU.subtract
                    )
                    nc.vector.scalar_tensor_tensor(
                        t2_t[:], ss2_t[:], c22, H_t[:], ALU.add, ALU.subtract
                    )
                    nc.vector.scalar_tensor_tensor(
                        num_t[:], G_t[:], c11, t1_t[:], ALU.add, ALU.mult
                    )
                    nc.vector.scalar_tensor_tensor(
                        den_t[:], H_t[:], c11, t2_t[:], ALU.add, ALU.mult
                    )
                    nc.vector.reciprocal_approx_fast(out=rec_t[:], in_=den_t[:])
                    col = 4 * img + t
                    nc.vector.scalar_tensor_tensor(
                        scr_t[:], num_t[:], 1.0, rec_t[:], ALU.mult, ALU.mult,
                        accum_out=acc_ss[:, col : col + 1],
                    )

            # ---------------- final reduction ----------------
            red = singles.tile([128, 5], F32, tag="red")
            nc.vector.reduce_sum(red[:, 0:1], acc_x2[:], axis=mybir.AxisListType.X)
            nc.vector.reduce_sum(red[:, 1:2], acc_y2[:], axis=mybir.AxisListType.X)
            nc.vector.reduce_sum(red[:, 2:3], acc_xy[:], axis=mybir.AxisListType.X)
            nc.vector.reduce_sum(red[:, 3:4], acc_ss[:], axis=mybir.AxisListType.X)
            nc.vector.tensor_copy(red[:, 4:5], acc_bce[:])
            ps_f = p1pool.tile([128, 512], F32, tag="p1")
            nc.tensor.matmul(ps_f[:1, 0:5], ones[:], red[:], start=True, stop=True)
            out_sb = singles.tile([1, 8], F32, tag="osb")
            nc.vector.memset(out_sb[:], 0.0)
            nc.vector.tensor_copy(out_sb[:, 0:5], ps_f[:1, 0:5])
            nc.sync.dma_start(out=out[:], in_=out_sb[:])

    if compile:
        nc.compile()
    return nc


_NC_CACHE = None


def _get_program():
    global _NC_CACHE
    if _NC_CACHE is None:
        _NC_CACHE = _build_program()
    return _NC_CACHE


def _make_in_maps(cover, wmed, wm_orig, wm_ext):
    sets = {
        "b1": _band_blocks(1.0),
        "b4": _band_blocks(4.0),
        "b2": _band_blocks(2.0),
    }
    sets["b1n"] = [(-b).astype(np.float16) for b in sets["b1"]]
    in_maps = []
    for c in range(N_CORES):
        sl = slice(c * B_LOC, (c + 1) * B_LOC)
        m = {
            "cover": np.ascontiguousarray(cover[sl]).astype(np.float16),
            "wmed": np.ascontiguousarray(wmed[sl]).astype(np.float16),
            "wm_orig": np.ascontiguousarray(wm_orig[sl]),
            "wm_ext": np.ascontiguousarray(wm_ext[sl]),
        }
        for nm, blocks in sets.items():
            for k in range(4):
                m[f"{nm}_{k}"] = blocks[k]
        in_maps.append(m)
    return in_maps


def _combine(results, epoch):
    tx2 = ty2 = txy = tss = tbce = 0.0
    for r in results:
        v = np.asarray(r["out"], dtype=np.float64).reshape(-1)
        tx2 += v[0]
        ty2 += v[1]
        txy += v[2]
        tss += v[3]
        tbce += v[4]
    n_pix = float(B * C * H * W)
    ml = (tx2 + ty2 - 2.0 * txy) / n_pix
    sv = tss / n_pix
    wl = -tbce / float(B * NB)
    epoch = int(epoch)
    if epoch <= CURRICULUM_EP:
        w_img, w_ssim = 0.05, 0.05
    else:
        progress = min(1.0, (epoch - CURRICULUM_EP) / 10.0)
        w_img = 0.05 + (LI - 0.05) * progress
        w_ssim = 0.05 + (LS - 0.05) * progress
    total = w_img * ml + w_ssim * (1.0 - sv) + LW * wl
    return (
        np.float32(total),
        np.float32(ml),
        np.float32(sv),
        np.float32(wl),
    )


def kernel(cover, wmed, wm_orig, wm_ext, epoch):
    from concourse.bass_utils import run_bass_kernel_spmd

    nc = _get_program()
    in_maps = _make_in_maps(
        np.asarray(cover, dtype=np.float32),
        np.asarray(wmed, dtype=np.float32),
        np.asarray(wm_orig, dtype=np.float32),
        np.asarray(wm_ext, dtype=np.float32),
    )
    res = run_bass_kernel_spmd(nc, in_maps, core_ids=list(range(N_CORES)))
    return _combine(res.results, epoch)


# revision 9
# speedup vs baseline: 1.1972x; 1.1972x over previous
"""v4: fp16, xp/xm folded into PE pass-1, minimal DVE phase A, batched tail.

Fields smoothed: {x, y, xy (taps x4 in pass1), s=x^2+y^2 (taps x2 in pass1)}.
Pass-1 accumulates xp-interm = pass1(x)+pass1(y), xm-interm = pass1(x)-pass1(y)
directly in PSUM using banded matmuls with +band / -band, so no elementwise
xp/xm ops exist at all.

Outputs per core: [sum_x2, sum_y2, sum_xy, sum_ssim, sum_bce];
host: mse_sum = sum_x2 + sum_y2 - 2*sum_xy.
"""

import sys

sys.path.insert(0, "/opt/trn_rl_repo")

import numpy as np

import concourse.bass as bass
import concourse.bacc as bacc
import concourse.mybir as mybir
from concourse.mybir import ActivationFunctionType as AF
from concourse.mybir import AluOpType as ALU
from concourse.tile import TileContext

F32 = mybir.dt.float32
F16 = mybir.dt.float16

B, C, H, W = 16, 3, 512, 512
NB = 1024
N_CORES = 8
B_LOC = B // N_CORES
N_IMG = B_LOC * C
C1 = 0.01 ** 2
C2 = 0.03 ** 2
CURRICULUM_EP = 12
LI, LS, LW = 0.5, 0.8, 3.0

OFFS = [0, 123, 251, 379]
NS = [512, 138, 138, 133]


def _gauss_1d():
    coords = np.arange(11, dtype=np.float32) - 5
    g = np.exp(-(coords ** 2) / (2 * 1.5 ** 2)).astype(np.float32)
    g = g / g.sum()
    return g.astype(np.float32)


def _tuned_f16_taps():
    """fp16 taps whose float64 sum is exactly 1 (tuned via the tiny edge
    taps); the linear/quadratic tap-sum mismatch in the SSIM covariance
    otherwise becomes a systematic bias."""
    g = _gauss_1d().astype(np.float64)
    t = g.astype(np.float16)
    for _ in range(20):
        r = 1.0 - t.astype(np.float64).sum()
        if abs(r) < 1e-9:
            break
        t[0] = np.float16(t[0] + r / 2)
        t[10] = np.float16(t[10] + r / 2)
    return t.astype(np.float32)


def _band_blocks(scale=1.0):
    """Band blocks for taps*scale (scale must be a power of two so the f16
    values stay exact)."""
    g = _tuned_f16_taps() * scale
    blocks = []
    for k in range(4):
        blk = np.zeros((128, NS[k]), dtype=np.float32)
        for r in range(128):
            h_in = 128 * k + r
            for j in range(NS[k]):
                h_out = OFFS[k] + j
                d = h_in - h_out + 5
                if 0 <= d <= 10:
                    blk[r, j] = g[d]
        blocks.append(blk.astype(np.float16))
    return blocks


# y-part of pass-1 for the "k=0" matmul only needs the tap-covered columns
NS_Y0 = 133


def _build_program(compile=True):
    nc = bacc.Bacc("TRN2", target_bir_lowering=False)

    cover = nc.declare_dram_parameter("cover", [B_LOC, C, H, W], F16, isOutput=False)
    wmed = nc.declare_dram_parameter("wmed", [B_LOC, C, H, W], F16, isOutput=False)
    wm_orig = nc.declare_dram_parameter("wm_orig", [B_LOC, NB], F32, isOutput=False)
    wm_ext = nc.declare_dram_parameter("wm_ext", [B_LOC, NB], F32, isOutput=False)
    # band sets: b1 (taps), b1n (-taps), b4 (4*taps), b2 (2*taps)
    band_names = ["b1", "b1n", "b4", "b2"]
    bands = {
        nm: [
            nc.declare_dram_parameter(f"{nm}_{k}", [128, NS[k]], F16, isOutput=False)
            for k in range(4)
        ]
        for nm in band_names
    }
    out = nc.declare_dram_parameter("out", [1, 8], F32, isOutput=True)

    c11 = 2.0 * C1
    c22 = 2.0 * C2

    with TileContext(nc) as tc:
        import contextlib

        with contextlib.ExitStack() as ctx:
            singles = ctx.enter_context(tc.tile_pool(name="singles", bufs=1))
            imgpool = ctx.enter_context(tc.tile_pool(name="img", bufs=2))
            fieldpool = ctx.enter_context(tc.tile_pool(name="field", bufs=3))
            itmpool = ctx.enter_context(tc.tile_pool(name="itm", bufs=3))
            cpool = ctx.enter_context(tc.tile_pool(name="pc", bufs=3))
            p1pool = ctx.enter_context(tc.tile_pool(name="psum1", bufs=2, space="PSUM"))
            p2pool = ctx.enter_context(tc.tile_pool(name="psum2", bufs=6, space="PSUM"))

            band_sb = {}
            for nm in band_names:
                tiles = []
                for k in range(4):
                    t = singles.tile([128, NS[k]], F16, tag=f"{nm}_{k}")
                    nc.sync.dma_start(out=t[:], in_=bands[nm][k][:])
                    tiles.append(t)
                band_sb[nm] = tiles
            ones = singles.tile([128, 1], F32, tag="ones")
            nc.vector.memset(ones[:], 1.0)

            acc_x2 = singles.tile([128, N_IMG], F32, tag="acc_x2")
            acc_y2 = singles.tile([128, N_IMG], F32, tag="acc_y2")
            acc_xy = singles.tile([128, N_IMG], F32, tag="acc_xy")
            acc_ss = singles.tile([128, 4 * N_IMG], F32, tag="acc_ss")
            acc_bce = singles.tile([128, 1], F32, tag="acc_bce")
            nc.vector.memset(acc_bce[:], 0.0)

            # ---------------- BCE ----------------
            o_t = singles.tile([B_LOC, NB], F32, tag="wmo")
            e_t = singles.tile([B_LOC, NB], F32, tag="wme")
            nc.sync.dma_start(out=o_t[:], in_=wm_orig[:])
            nc.sync.dma_start(out=e_t[:], in_=wm_ext[:])
            l1 = singles.tile([B_LOC, NB], F32, tag="l1")
            l2 = singles.tile([B_LOC, NB], F32, tag="l2")
            om = singles.tile([B_LOC, NB], F32, tag="om")
            d12 = singles.tile([B_LOC, NB], F32, tag="d12")
            m1 = singles.tile([B_LOC, NB], F32, tag="m1")
            nc.scalar.activation(l1[:], e_t[:], AF.Ln)
            nc.vector.tensor_scalar(om[:], e_t[:], -1.0, 1.0, ALU.mult, ALU.add)
            nc.scalar.activation(l2[:], om[:], AF.Ln)
            nc.vector.tensor_tensor(d12[:], l1[:], l2[:], ALU.subtract)
            nc.vector.tensor_tensor(m1[:], o_t[:], d12[:], ALU.mult)
            nc.vector.scalar_tensor_tensor(
                m1[:], m1[:], 0.0, l2[:], ALU.add, ALU.add,
                accum_out=acc_bce[:B_LOC, 0:1],
            )

            # ---------------- main loop ----------------
            for img in range(N_IMG):
                b, ch = divmod(img, C)
                x16 = fieldpool.tile([128, 2048], F16, tag="x16")
                y16 = fieldpool.tile([128, 2048], F16, tag="y16")
                x2 = fieldpool.tile([128, 2048], F16, tag="x2")
                y2 = fieldpool.tile([128, 2048], F16, tag="y2")
                xy = fieldpool.tile([128, 2048], F16, tag="xy")
                s_f = fieldpool.tile([128, 2048], F16, tag="s")
                src_x = wmed[b, ch].rearrange("(t p) w -> p t w", p=128)
                src_y = cover[b, ch].rearrange("(t p) w -> p t w", p=128)
                nc.sync.dma_start(out=x16[:].rearrange("p (t w) -> p t w", t=4), in_=src_x)
                nc.sync.dma_start(out=y16[:].rearrange("p (t w) -> p t w", t=4), in_=src_y)
                nc.scalar.activation(
                    x2[:], x16[:], AF.Square, accum_out=acc_x2[:, img : img + 1]
                )
                nc.scalar.activation(
                    y2[:], y16[:], AF.Square, accum_out=acc_y2[:, img : img + 1]
                )
                nc.vector.scalar_tensor_tensor(
                    xy[:], x16[:], 1.0, y16[:], ALU.mult, ALU.mult,
                    accum_out=acc_xy[:, img : img + 1],
                )
                nc.gpsimd.tensor_tensor(s_f[:], x2[:], y2[:], ALU.add)

                # pass 1 -> four fp16 interms: xp=(x+y), xm=(x-y), xy*4, s*2
                itm_xp = itmpool.tile([128, 2048], F16, tag="itm_xp")
                itm_xm = itmpool.tile([128, 2048], F16, tag="itm_xm")
                itm_xy = itmpool.tile([128, 2048], F16, tag="itm_xy")
                itm_s = itmpool.tile([128, 2048], F16, tag="itm_s")

                for m in range(4):
                    # xp / xm: accumulate x and +/-y matmuls in one psum tile
                    for itm, yband in ((itm_xp, "b1"), (itm_xm, "b1n")):
                        ps = p1pool.tile([128, 512], F32, tag="p1")
                        for k in range(4):
                            lhsT = x16[:, k * 512 + m * 128 : k * 512 + m * 128 + 128]
                            nc.tensor.matmul(
                                ps[:, OFFS[k] : OFFS[k] + NS[k]],
                                lhsT,
                                band_sb["b1"][k][:],
                                start=(k == 0),
                                stop=False,
                            )
                        for k in range(4):
                            lhsT = y16[:, k * 512 + m * 128 : k * 512 + m * 128 + 128]
                            n_k = NS_Y0 if k == 0 else NS[k]
                            nc.tensor.matmul(
                                ps[:, OFFS[k] : OFFS[k] + n_k],
                                lhsT,
                                band_sb[yband][k][:, 0:n_k],
                                start=False,
                                stop=(k == 3),
                            )
                        nc.any.tensor_copy(itm[:, m * 512 : (m + 1) * 512], ps[:])
                    for itm, F_t, bnm in (
                        (itm_xy, xy, "b4"),
                        (itm_s, s_f, "b2"),
                    ):
                        ps = p1pool.tile([128, 512], F32, tag="p1")
                        for k in range(4):
                            lhsT = F_t[:, k * 512 + m * 128 : k * 512 + m * 128 + 128]
                            nc.tensor.matmul(
                                ps[:, OFFS[k] : OFFS[k] + NS[k]],
                                lhsT,
                                band_sb[bnm][k][:],
                                start=(k == 0),
                                stop=(k == 3),
                            )
                        nc.any.tensor_copy(itm[:, m * 512 : (m + 1) * 512], ps[:])

                # pass 2 + per-chunk phase C
                for t in range(4):
                    sm_ps = []
                    for itm in (itm_xp, itm_xm, itm_xy, itm_s):
                        ps = p2pool.tile([128, 512], F32, tag="p2")
                        sm_ps.append(ps)
                        for k in range(4):
                            lhsT = itm[:, k * 512 + t * 128 : k * 512 + t * 128 + 128]
                            nc.tensor.matmul(
                                ps[:, OFFS[k] : OFFS[k] + NS[k]],
                                lhsT,
                                band_sb["b1"][k][:],
                                start=(k == 0),
                                stop=(k == 3),
                            )
                    sp_t, smm_t, sm4_t, ss2_t = sm_ps

                    P_t = cpool.tile([128, 512], F32, tag="P")
                    Q_t = cpool.tile([128, 512], F32, tag="Q")
                    G_t = cpool.tile([128, 512], F32, tag="G")
                    H_t = cpool.tile([128, 512], F32, tag="Hh")
                    t1_t = cpool.tile([128, 512], F16, tag="t1")
                    t2_t = cpool.tile([128, 512], F16, tag="t2")
                    num_t = cpool.tile([128, 512], F16, tag="num")
                    den_t = cpool.tile([128, 512], F32, tag="den")
                    rec_t = cpool.tile([128, 512], F32, tag="rec")
                    scr_t = cpool.tile([128, 512], F32, tag="scr")

                    nc.scalar.activation(P_t[:], sp_t[:], AF.Square)
                    nc.scalar.activation(Q_t[:], smm_t[:], AF.Square)
                    nc.gpsimd.tensor_tensor(G_t[:], P_t[:], Q_t[:], ALU.subtract)
                    nc.gpsimd.tensor_tensor(H_t[:], P_t[:], Q_t[:], ALU.add)
                    nc.vector.scalar_tensor_tensor(
                        t1_t[:], sm4_t[:], c22, G_t[:], ALU.add, ALU.subtract
                    )
                    nc.vector.scalar_tensor_tensor(
                        t2_t[:], ss2_t[:], c22, H_t[:], ALU.add, ALU.subtract
                    )
                    nc.vector.scalar_tensor_tensor(
                        num_t[:], G_t[:], c11, t1_t[:], ALU.add, ALU.mult
                    )
                    nc.vector.scalar_tensor_tensor(
                        den_t[:], H_t[:], c11, t2_t[:], ALU.add, ALU.mult
                    )
                    nc.vector.reciprocal_approx_fast(out=rec_t[:], in_=den_t[:])
                    col = 4 * img + t
                    nc.vector.scalar_tensor_tensor(
                        scr_t[:], num_t[:], 1.0, rec_t[:], ALU.mult, ALU.mult,
                        accum_out=acc_ss[:, col : col + 1],
                    )

            # ---------------- final reduction ----------------
            red = singles.tile([128, 5], F32, tag="red")
            nc.vector.reduce_sum(red[:, 0:1], acc_x2[:], axis=mybir.AxisListType.X)
            nc.vector.reduce_sum(red[:, 1:2], acc_y2[:], axis=mybir.AxisListType.X)
            nc.vector.reduce_sum(red[:, 2:3], acc_xy[:], axis=mybir.AxisListType.X)
            nc.vector.reduce_sum(red[:, 3:4], acc_ss[:], axis=mybir.AxisListType.X)
            nc.vector.tensor_copy(red[:, 4:5], acc_bce[:])
            ps_f = p1pool.tile([128, 512], F32, tag="p1")
            nc.tensor.matmul(ps_f[:1, 0:5], ones[:], red[:], start=True, stop=True)
            out_sb = singles.tile([1, 8], F32, tag="osb")
            nc.vector.memset(out_sb[:], 0.0)
            nc.vector.tensor_copy(out_sb[:, 0:5], ps_f[:1, 0:5])
            nc.sync.dma_start(out=out[:], in_=out_sb[:])

    if compile:
        nc.compile()
    return nc


_NC_CACHE = None


def _get_program():
    global _NC_CACHE
    if _NC_CACHE is None:
        _NC_CACHE = _build_program()
    return _NC_CACHE


def _make_in_maps(cover, wmed, wm_orig, wm_ext):
    sets = {
        "b1": _band_blocks(1.0),
        "b4": _band_blocks(4.0),
        "b2": _band_blocks(2.0),
    }
    sets["b1n"] = [(-b).astype(np.float16) for b in sets["b1"]]
    in_maps = []
    for c in range(N_CORES):
        sl = slice(c * B_LOC, (c + 1) * B_LOC)
        m = {
            "cover": np.ascontiguousarray(cover[sl]).astype(np.float16),
            "wmed": np.ascontiguousarray(wmed[sl]).astype(np.float16),
            "wm_orig": np.ascontiguousarray(wm_orig[sl]),
            "wm_ext": np.ascontiguousarray(wm_ext[sl]),
        }
        for nm, blocks in sets.items():
            for k in range(4):
                m[f"{nm}_{k}"] = blocks[k]
        in_maps.append(m)
    return in_maps


def _combine(results, epoch):
    tx2 = ty2 = txy = tss = tbce = 0.0
    for r in results:
        v = np.asarray(r["out"], dtype=np.float64).reshape(-1)
        tx2 += v[0]
        ty2 += v[1]
        txy += v[2]
        tss += v[3]
        tbce += v[4]
    n_pix = float(B * C * H * W)
    ml = (tx2 + ty2 - 2.0 * txy) / n_pix
    sv = tss / n_pix
    wl = -tbce / float(B * NB)
    epoch = int(epoch)
    if epoch <= CURRICULUM_EP:
        w_img, w_ssim = 0.05, 0.05
    else:
        progress = min(1.0, (epoch - CURRICULUM_EP) / 10.0)
        w_img = 0.05 + (LI - 0.05) * progress
        w_ssim = 0.05 + (LS - 0.05) * progress
    total = w_img * ml + w_ssim * (1.0 - sv) + LW * wl
    return (
        np.float32(total),
        np.float32(ml),
        np.float32(sv),
        np.float32(wl),
    )


def kernel(cover, wmed, wm_orig, wm_ext, epoch):
    from concourse.bass_utils import run_bass_kernel_spmd

    nc = _get_program()
    in_maps = _make_in_maps(
        np.asarray(cover, dtype=np.float32),
        np.asarray(wmed, dtype=np.float32),
        np.asarray(wm_orig, dtype=np.float32),
        np.asarray(wm_ext, dtype=np.float32),
    )
    res = run_bass_kernel_spmd(nc, in_maps, core_ids=list(range(N_CORES)))
    return _combine(res.results, epoch)
